# revision 46
# baseline (speedup 1.0000x reference)
"""ConvTranspose2d(64->64,k4,s2,p1) + MaxPool2(2) + Hardtanh + spatial mean + tanh.

Full inputs: x[32,64,64,64] f32, w[64,64,4,4] f32, b[64] f32 -> out [32,64,1,1] f32.
Sharded batch-wise over 8 NeuronCores (4 batches/core), SPMD, no collectives.

Math: with stride 2 / k=4 / pad 1, conv-transpose output y[2m+pp, 2n+pq]
(parity class (pp,pq) in {0,1}^2) is a 2x2-tap stride-1 conv over x:
  y[2m+pp, 2n+pq] = sum_{c,dh,dw} x[c, m+pp+dh-1, n+pq+dw-1] * w[c,:,kh,kw]
  with kh=(3-pp)-2dh, kw=(3-pq)-2dw.
MaxPool(2,2) output [m,n] = max over the 4 parity maps at [m,n] (+bias, same
for all four). clip() is monotone so it commutes with max; the per-channel
bias folds into the clip bounds (clip(v+b,-1,1) = clip_{[-1-b,1-b]}(v)+b) and
the final mean/bias/tanh fuse into one scalar-engine activation:
  out = tanh(clipped_sum/4096 + b).

Host-side prep (numpy, not on the device clock): zero-pad x to 66x66, stack
the dh=0 / dh=1 row-shifted copies on the partition axis (so matmul K=128 =
2 taps x 64 input channels), pre-arrange the 8 stationary weight tiles, and
round both to tf32 (float32r) which streams through the PE at 1 column/cycle
(true fp32 is 4x slower). Each SBUF tile then has exactly ONE DMA producer
(this compile path allows a single semaphore wait per instruction).
"""

import os

import numpy as np

import concourse.bass as bass
import concourse.mybir as mybir
import concourse.tile as tile

B, C, H, W = 32, 64, 64, 64
NCORES = 8
BPC = B // NCORES  # batches per core
PD = 66            # padded spatial dim
NCHUNK = 8         # spatial chunks per batch (each = 8 pooled rows = 512 values)
F32 = mybir.dt.float32
MMDT = mybir.dt.bfloat16   # conv operand dtype (1 col/cycle on PE, FWL)
PPDT = mybir.dt.bfloat16   # post-max pipeline dtype (2x/4x DVE modes)
ALU = mybir.AluOpType
GCHUNK = int(os.environ.get("GCHUNK", "2"))  # chunks per repack group
XSPLIT = os.environ.get("XSPLIT", "1") == "1"  # split x loads into 2 DMAs
TAILENG = os.environ.get("TAILENG", "vector")  # engine for clip/sum tail
CDDT = mybir.dt.bfloat16 if os.environ.get("CDBF16", "1") == "1" else mybir.dt.float32
MAXMODE = os.environ.get("MAXMODE", "psum")  # psum | sbuf
DVECOPY = int(os.environ.get("DVECOPY", "4"))  # in sbuf mode: every Nth copy on DVE

# stationary slots; None = zeros (fp32r matmuls must write PSUM partition 0,
# so upper-half-only classes become M=128 with a zeroed lower half)
# [13,14] = AB pair, [9,10] = CD pair, 15/11 = lower singles,
# [None,12] / [None,8] = upper-half classes as zero-padded pairs
W_SLOTS = [13, 14, 9, 10, 15, 11, None, 12, None, 8]


def _legalize_waits(nc):
    """walrus codegen allows one sync-wait per instruction; hoist extras onto
    same-engine NoOps inserted immediately before."""
    import bass_rust
    ctr = 0
    for f in nc.m.functions:
        for blk in f.blocks:
            insts = blk.instructions
            out = []
            changed = False
            for inst in insts:
                si = inst.sync_info
                if si is not None and len(si.on_wait) > 1:
                    waits = list(si.on_wait)
                    for w in waits[:-1]:
                        nop = bass_rust.InstNoOp(
                            name=f"I-waitfix-{ctr}", ins=[], outs=[])
                        ctr += 1
                        nop.engine = inst.engine
                        nop.sync_info = mybir.SyncInfo(on_wait=[w], on_update=[])
                        out.append(nop)
                    inst.sync_info = mybir.SyncInfo(
                        on_wait=[waits[-1]], on_update=list(si.on_update))
                    changed = True
                out.append(inst)
            if changed:
                insts.clear()
                insts.extend(out)
    return ctr


def build_nc(legalize=True, loop_n=None):
    """loop_n: if set, repeat the whole body loop_n times on-device via a
    hardware For_i loop (used only for wall-clock timing of the kernel)."""
    nc = bass.Bass("TRN2", target_bir_lowering=False, debug=False)
    xp_d = nc.dram_tensor("xp", [BPC, 128, PD, PD], MMDT, kind="ExternalInput").ap()
    ws_d = nc.dram_tensor("ws", [128, len(W_SLOTS), 64], MMDT, kind="ExternalInput").ap()
    cs_d = nc.dram_tensor("cs", [128, 3], F32, kind="ExternalInput").ap()
    out_d = nc.dram_tensor("out", [BPC, C], F32, kind="ExternalOutput").ap()

    with tile.TileContext(nc) as tc:
        if loop_n is None:
            _body(tc, out_d, xp_d, ws_d, cs_d)
        else:
            with tc.For_i(0, loop_n, 1):
                _body(tc, out_d, xp_d, ws_d, cs_d)
    if legalize:
        # CoreSim can't execute the synthetic NoOps; only the HW compile
        # path needs them (sync-only rewrite, data flow unchanged).
        _legalize_waits(nc)
    return nc


def _body(tc, out_d, xp_d, ws_d, cs_d):
    nc = tc.nc
    import contextlib
    ctx = contextlib.ExitStack()
    with ctx:
        const_pool = ctx.enter_context(tc.tile_pool(name="const", bufs=1))
        xpool = ctx.enter_context(tc.tile_pool(name="xp", bufs=1))
        qpool = ctx.enter_context(tc.tile_pool(name="qp", bufs=3))
        cdpool = ctx.enter_context(tc.tile_pool(name="cdp", bufs=6))
        lrpool = ctx.enter_context(tc.tile_pool(name="lrp", bufs=3))
        spool = ctx.enter_context(tc.tile_pool(name="sp", bufs=2))
        pspool = ctx.enter_context(tc.tile_pool(name="ps", bufs=8, space="PSUM"))

        w_all = const_pool.tile([128, len(W_SLOTS), 64], MMDT, tag="w_all")
        nc.sync.dma_start(w_all[:, :, :], ws_d)
        cs = const_pool.tile([128, 3], F32, tag="cs")
        nc.sync.dma_start(cs[:, :], cs_d)
        hi, lo, bb = cs[:, 0:1], cs[:, 1:2], cs[:, 2:3]

        xt = []
        for bi in range(BPC):
            t = xpool.tile([128, PD, PD], MMDT, tag=f"x{bi}")
            if XSPLIT:
                nc.sync.dma_start(t[:, 0:34, :], xp_d[bi][:, 0:34, :])
                nc.sync.dma_start(t[:, 34:PD, :], xp_d[bi][:, 34:PD, :])
            else:
                nc.sync.dma_start(t[:, :, :], xp_d[bi])
            xt.append(t)

        def wst1(s):
            return w_all[:, s, :]

        def wst2(s):
            return w_all[:, s:s + 2, :].rearrange("p a o -> p (a o)")

        # per chunk-batch: bank AB = [c00 | c01], bank CD = [c10 | c11]
        # moving AP rows m0+pp : m0+pp+8, cols cw : cw+64 (cw = pq+dw)
        # modes: pair = M=128 start; lower = M=64 into partitions 0:64;
        # upperz = M=128 zero-padded lower, stop
        AB_MMS = [(1, 0, "pair"), (0, 4, "lower"), (2, 6, "upperz")]
        CD_MMS = [(1, 2, "pair"), (0, 5, "lower"), (2, 8, "upperz")]

        inv_n = 1.0 / (64.0 * 64.0)

        GC = GCHUNK  # chunks per repack group
        ngroups = NCHUNK // GC
        for p in range(2):  # batch pairs
            b0, b1 = 2 * p, 2 * p + 1
            acc = spool.tile([128, ngroups], F32, tag="acc")
            for g in range(ngroups):
                # qq[:, k, i, :] = [max(c00,c10) | max(c01,c11)], batch i
                qq = qpool.tile([128, GC, 2, 512], PPDT, tag="qq")
                for kk in range(GC):
                    m0 = 8 * (g * GC + kk)
                    for i, bbatch in enumerate((b0, b1)):
                        t = xt[bbatch]
                        psAB = pspool.tile([128, 512], F32, tag="ps")
                        psCD = pspool.tile([128, 512], F32, tag="ps")
                        for ps, mms, pp in ((psAB, AB_MMS, 0), (psCD, CD_MMS, 1)):
                            r0 = m0 + pp
                            for cw, s, mode in mms:
                                rhs = t[:, r0:r0 + 8, cw:cw + 64]
                                # partial-partition accumulation groups confuse
                                # the sim's group tracker; pending-zero still
                                # validates the data -> skip the group check.
                                if mode == "pair":
                                    nc.tensor.matmul(
                                        ps[:, :], wst2(s), rhs,
                                        start=True, stop=False,
                                        skip_group_check=True)
                                elif mode == "lower":
                                    nc.tensor.matmul(
                                        ps[0:64, :], wst1(s), rhs,
                                        start=False, stop=False,
                                        skip_group_check=True)
                                else:  # upperz
                                    nc.tensor.matmul(
                                        ps[:, :], wst2(s), rhs,
                                        start=False, stop=True,
                                        skip_group_check=True)
                        # DVE may read only one PSUM operand. Two schemes:
                        # MAXMODE psum: ScalarE copies CD; DVE maxes
                        #   PSUM(AB) x SBUF(cd) at 1x.
                        # MAXMODE sbuf: both banks copied to bf16 SBUF
                        #   (ScalarE mostly, DVE for every 4th) so the max
                        #   runs at bf16 2x on DVE.
                        cbidx = (g * GC + kk) * 2 + i
                        if MAXMODE == "psum":
                            cd = cdpool.tile([128, 512], CDDT, tag="cd")
                            nc.scalar.copy(cd[:, :], psCD[:, :])
                            nc.vector.tensor_tensor(
                                qq[:, kk, i, :], psAB[:, :], cd[:, :], ALU.max)
                        else:
                            ca = cdpool.tile([128, 512], CDDT, tag="ca")
                            cd = cdpool.tile([128, 512], CDDT, tag="cd")
                            if cbidx % DVECOPY == DVECOPY - 1:
                                nc.vector.tensor_copy(ca[:, :], psAB[:, :])
                            else:
                                nc.scalar.copy(ca[:, :], psAB[:, :])
                            nc.scalar.copy(cd[:, :], psCD[:, :])
                            nc.vector.tensor_tensor(
                                qq[:, kk, i, :], ca[:, :], cd[:, :], ALU.max)
                # repack group to 128-lane batch-pair layout
                L = lrpool.tile([128, GC, 512], PPDT, tag="L")
                R = lrpool.tile([128, GC, 512], PPDT, tag="R")
                nc.sync.dma_start(L[0:64, :, :], qq[0:64, :, 0, :])
                nc.sync.dma_start(L[64:128, :, :], qq[0:64, :, 1, :])
                nc.sync.dma_start(R[0:64, :, :], qq[64:128, :, 0, :])
                nc.sync.dma_start(R[64:128, :, :], qq[64:128, :, 1, :])
                # clip R in place; then max(min(L,hi), R) in place over R;
                # accum_out = per-channel sum for this group
                te = getattr(nc, TAILENG)
                te.tensor_scalar(
                    R[:, :, :], R[:, :, :], hi, lo, ALU.min, ALU.max)
                te.scalar_tensor_tensor(
                    R[:, :, :], L[:, :, :], hi, R[:, :, :], ALU.min, ALU.max,
                    accum_out=acc[:, g:g + 1])
            S = spool.tile([128, 1], F32, tag="S")
            if ngroups > 1:
                nc.vector.tensor_reduce(
                    S[:, :], acc[:, :], mybir.AxisListType.X, ALU.add)
            else:
                S = acc
            T = spool.tile([128, 1], F32, tag="T")
            nc.scalar.activation(
                T[:, :], S[:, :], mybir.ActivationFunctionType.Tanh,
                bias=bb, scale=inv_n)
            nc.sync.dma_start(out_d[2 * p:2 * p + 2, :], T[:, :])


def _round_tf32(a: np.ndarray) -> np.ndarray:
    """Round fp32 to tf32 (10-bit mantissa, round-to-nearest-even)."""
    u = np.ascontiguousarray(a, dtype=np.float32).view(np.uint32)
    bias = np.uint32(0xFFF) + ((u >> np.uint32(13)) & np.uint32(1))
    return ((u + bias) & np.uint32(0xFFFFE000)).view(np.float32)


def prep_core_inputs(x, w, b):
    """Host-side prep: pad/duplicate x, stationary-arrange w, fold b."""
    import ml_dtypes
    mmnp = ml_dtypes.bfloat16
    x = np.asarray(x, dtype=np.float32)
    w = np.asarray(w, dtype=np.float32)
    b = np.asarray(b, dtype=np.float32)

    ws = np.zeros((128, len(W_SLOTS), 64), np.float32)
    for s, j in enumerate(W_SLOTS):
        if j is None:
            continue
        kh, kw = j // 4, j % 4
        ws[0:64, s, :] = w[:, :, kh, kw]
        ws[64:128, s, :] = w[:, :, kh - 2, kw]
    ws = ws.astype(mmnp)

    cs = np.zeros((128, 3), np.float32)
    bd = np.concatenate([b, b])
    cs[:, 0] = 1.0 - bd
    cs[:, 1] = -1.0 - bd
    cs[:, 2] = bd

    in_maps = []
    for i in range(NCORES):
        xs = x[i * BPC:(i + 1) * BPC]
        xp = np.zeros((BPC, 128, PD, PD), np.float32)
        xp[:, 0:64, 1:65, 1:65] = xs    # dh=0 taps: P[r,s] = x[r-1,s-1]
        xp[:, 64:128, 0:64, 1:65] = xs  # dh=1 taps: shifted up one row
        in_maps.append({"xp": xp.astype(mmnp), "ws": ws, "cs": cs})
    return in_maps


class Runner:
    """Builds the 8-core shard_map'd executable once; callable many times
    (mirrors concourse.bass2jax.run_bass_via_pjrt)."""

    def __init__(self, nc=None):
        import jax
        from jax.sharding import Mesh, PartitionSpec, NamedSharding
        try:
            from jax.experimental.shard_map import shard_map
        except ImportError:
            from jax import shard_map
        from concourse.bass2jax import (
            _bass_exec_p, partition_id_tensor, install_neuronx_cc_hook)

        install_neuronx_cc_hook()
        self.nc = nc = nc if nc is not None else build_nc()
        pname = nc.partition_id_tensor.name if nc.partition_id_tensor else None
        in_names, out_names, out_avals, zero_outs = [], [], [], []
        for alloc in nc.m.functions[0].allocations:
            if not isinstance(alloc, mybir.MemoryLocationSet):
                continue
            name = alloc.memorylocations[0].name
            if alloc.kind == "ExternalInput":
                if name != pname:
                    in_names.append(name)
            elif alloc.kind == "ExternalOutput":
                out_names.append(name)
                shape = tuple(alloc.tensor_shape)
                dtype = mybir.dt.np(alloc.dtype)
                out_avals.append(jax.core.ShapedArray(shape, dtype))
                zero_outs.append(np.zeros(shape, dtype))
        self.in_names = list(in_names)
        self.out_names = out_names
        self.zero_outs = zero_outs
        n_params, n_outs = len(in_names), len(out_names)
        all_in = in_names + out_names + ([pname] if pname else [])

        def _body(*args):
            operands = list(args)
            if pname:
                operands.append(partition_id_tensor())
            return tuple(_bass_exec_p.bind(
                *operands,
                out_avals=tuple(out_avals),
                in_names=tuple(all_in),
                out_names=tuple(out_names),
                lowering_input_output_aliases=(),
                sim_require_finite=True,
                sim_require_nnan=True,
                nc=nc,
            ))

        devices = jax.devices()[:NCORES]
        self.mesh = Mesh(np.asarray(devices), ("core",))
        self.spec = PartitionSpec("core")
        self.sharding = NamedSharding(self.mesh, self.spec)
        in_specs = (self.spec,) * (n_params + n_outs)
        out_specs = (self.spec,) * n_outs
        self.fn = jax.jit(
            shard_map(_body, mesh=self.mesh, in_specs=in_specs,
                      out_specs=out_specs, check_rep=False),
            donate_argnums=tuple(range(n_params, n_params + n_outs)),
            keep_unused=True,
        )
        self._jax = jax

    def stage_inputs(self, in_maps):
        concat = [np.concatenate([np.asarray(m[n]) for m in in_maps], axis=0)
                  for n in self.in_names]
        return [self._jax.device_put(a, self.sharding) for a in concat]

    def __call__(self, staged):
        zeros = [np.zeros((NCORES * z.shape[0], *z.shape[1:]), z.dtype)
                 for z in self.zero_outs]
        return self.fn(*staged, *zeros)

    def run(self, in_maps):
        outs = self(self.stage_inputs(in_maps))
        return [
            {n: np.asarray(outs[i]).reshape(NCORES, *self.zero_outs[i].shape)[c]
             for i, n in enumerate(self.out_names)}
            for c in range(NCORES)
        ]


def kernel(x: np.ndarray, w: np.ndarray, b: np.ndarray) -> np.ndarray:
    in_maps = prep_core_inputs(x, w, b)
    r = Runner()
    res = r.run(in_maps)
    out = np.concatenate([res[i]["out"] for i in range(NCORES)], axis=0)
    return out.reshape(B, C, 1, 1).astype(np.float32)


if __name__ == "__main__":
    rng = np.random.default_rng(0)
    x = rng.standard_normal((B, C, H, W), dtype=np.float32)
    w = rng.standard_normal((C, C, 4, 4), dtype=np.float32) * 0.05
    b = rng.standard_normal((C,), dtype=np.float32) * 0.05
    print(kernel(x, w, b).shape)


# revision 49
# speedup vs baseline: 1.0286x; 1.0286x over previous
"""ConvTranspose2d(64->64,k4,s2,p1) + MaxPool2(2) + Hardtanh + spatial mean + tanh.

Full inputs: x[32,64,64,64] f32, w[64,64,4,4] f32, b[64] f32 -> out [32,64,1,1] f32.
Sharded batch-wise over 8 NeuronCores (4 batches/core), SPMD, no collectives.

Math: with stride 2 / k=4 / pad 1, conv-transpose output y[2m+pp, 2n+pq]
(parity class (pp,pq) in {0,1}^2) is a 2x2-tap stride-1 conv over x:
  y[2m+pp, 2n+pq] = sum_{c,dh,dw} x[c, m+pp+dh-1, n+pq+dw-1] * w[c,:,kh,kw]
  with kh=(3-pp)-2dh, kw=(3-pq)-2dw.
MaxPool(2,2) output [m,n] = max over the 4 parity maps at [m,n] (+bias, same
for all four). clip() is monotone so it commutes with max; the per-channel
bias folds into the clip bounds (clip(v+b,-1,1) = clip_{[-1-b,1-b]}(v)+b) and
the final mean/bias/tanh fuse into one scalar-engine activation:
  out = tanh(clipped_sum/4096 + b).

Host-side prep (numpy, not on the device clock): zero-pad x to 66x66, stack
the dh=0 / dh=1 row-shifted copies on the partition axis (so matmul K=128 =
2 taps x 64 input channels), pre-arrange the stationary weight tiles, and
cast both to bf16, which streams through the PE at 1 column/cycle (true fp32
is 4x slower) with fast weight loads. Each SBUF tile has a single DMA
producer where possible; _legalize_waits splits any remaining multi-wait
instructions (this compile path allows one semaphore wait per instruction).
"""

import os

import numpy as np

import concourse.bass as bass
import concourse.mybir as mybir
import concourse.tile as tile

B, C, H, W = 32, 64, 64, 64
NCORES = 8
BPC = B // NCORES  # batches per core
PD = 66            # padded spatial dim
NCHUNK = 8         # spatial chunks per batch (each = 8 pooled rows = 512 values)
F32 = mybir.dt.float32
MMDT = mybir.dt.bfloat16   # conv operand dtype (1 col/cycle on PE, FWL)
PPDT = mybir.dt.bfloat16   # post-max pipeline dtype (2x/4x DVE modes)
ALU = mybir.AluOpType
GCHUNK = int(os.environ.get("GCHUNK", "2"))  # chunks per repack group
XSPLIT = os.environ.get("XSPLIT", "1") == "1"  # split x loads into 2 DMAs
TAILENG = os.environ.get("TAILENG", "vector")  # engine for clip/sum tail
CDDT = mybir.dt.bfloat16 if os.environ.get("CDBF16", "1") == "1" else mybir.dt.float32
MAXMODE = os.environ.get("MAXMODE", "psum")  # psum | sbuf
DVECOPY = int(os.environ.get("DVECOPY", "4"))  # in sbuf mode: every Nth copy on DVE
PEONLY = os.environ.get("PEONLY", "0") == "1"  # timing diagnostic: drop post-processing

# stationary slots; None = zeros (fp32r matmuls must write PSUM partition 0,
# so upper-half-only classes become M=128 with a zeroed lower half)
# [13,14] = AB pair, [9,10] = CD pair, 15/11 = lower singles,
# [None,12] / [None,8] = upper-half classes as zero-padded pairs
W_SLOTS = [13, 14, 9, 10, 15, 11, None, 12, None, 8]


def _legalize_waits(nc):
    """walrus codegen allows one sync-wait per instruction; hoist extras onto
    same-engine NoOps inserted immediately before."""
    import bass_rust
    ctr = 0
    for f in nc.m.functions:
        for blk in f.blocks:
            insts = blk.instructions
            out = []
            changed = False
            for inst in insts:
                si = inst.sync_info
                if si is not None and len(si.on_wait) > 1:
                    waits = list(si.on_wait)
                    for w in waits[:-1]:
                        nop = bass_rust.InstNoOp(
                            name=f"I-waitfix-{ctr}", ins=[], outs=[])
                        ctr += 1
                        nop.engine = inst.engine
                        nop.sync_info = mybir.SyncInfo(on_wait=[w], on_update=[])
                        out.append(nop)
                    inst.sync_info = mybir.SyncInfo(
                        on_wait=[waits[-1]], on_update=list(si.on_update))
                    changed = True
                out.append(inst)
            if changed:
                insts.clear()
                insts.extend(out)
    return ctr


def build_nc(legalize=True, loop_n=None):
    """loop_n: if set, repeat the whole body loop_n times on-device via a
    hardware For_i loop (used only for wall-clock timing of the kernel)."""
    nc = bass.Bass("TRN2", target_bir_lowering=False, debug=False)
    xp_d = nc.dram_tensor("xp", [BPC, 128, PD, PD], MMDT, kind="ExternalInput").ap()
    ws_d = nc.dram_tensor("ws", [128, len(W_SLOTS), 64], MMDT, kind="ExternalInput").ap()
    cs_d = nc.dram_tensor("cs", [128, 3], F32, kind="ExternalInput").ap()
    out_d = nc.dram_tensor("out", [BPC, C], F32, kind="ExternalOutput").ap()

    with tile.TileContext(nc) as tc:
        if loop_n is None:
            _body(tc, out_d, xp_d, ws_d, cs_d)
        else:
            with tc.For_i(0, loop_n, 1):
                _body(tc, out_d, xp_d, ws_d, cs_d)
    if legalize:
        # CoreSim can't execute the synthetic NoOps; only the HW compile
        # path needs them (sync-only rewrite, data flow unchanged).
        _legalize_waits(nc)
    return nc


def _body(tc, out_d, xp_d, ws_d, cs_d):
    nc = tc.nc
    import contextlib
    ctx = contextlib.ExitStack()
    with ctx:
        const_pool = ctx.enter_context(tc.tile_pool(name="const", bufs=1))
        xpool = ctx.enter_context(tc.tile_pool(name="xp", bufs=1))
        qpool = ctx.enter_context(tc.tile_pool(name="qp", bufs=3))
        cdpool = ctx.enter_context(tc.tile_pool(name="cdp", bufs=6))
        lrpool = ctx.enter_context(tc.tile_pool(name="lrp", bufs=3))
        spool = ctx.enter_context(tc.tile_pool(name="sp", bufs=2))
        pspool = ctx.enter_context(tc.tile_pool(name="ps", bufs=8, space="PSUM"))

        w_all = const_pool.tile([128, len(W_SLOTS), 64], MMDT, tag="w_all")
        nc.sync.dma_start(w_all[:, :, :], ws_d)
        cs = const_pool.tile([128, 3], F32, tag="cs")
        nc.sync.dma_start(cs[:, :], cs_d)
        hi, lo, bb = cs[:, 0:1], cs[:, 1:2], cs[:, 2:3]

        xt = []
        for bi in range(BPC):
            t = xpool.tile([128, PD, PD], MMDT, tag=f"x{bi}")
            if XSPLIT:
                nc.sync.dma_start(t[:, 0:34, :], xp_d[bi][:, 0:34, :])
                nc.sync.dma_start(t[:, 34:PD, :], xp_d[bi][:, 34:PD, :])
            else:
                nc.sync.dma_start(t[:, :, :], xp_d[bi])
            xt.append(t)

        def wst1(s):
            return w_all[:, s, :]

        def wst2(s):
            return w_all[:, s:s + 2, :].rearrange("p a o -> p (a o)")

        # per chunk-batch: bank AB = [c00 | c01], bank CD = [c10 | c11]
        # moving AP rows m0+pp : m0+pp+8, cols cw : cw+64 (cw = pq+dw)
        # modes: pair = M=128 start; lower = M=64 into partitions 0:64;
        # upperz = M=128 zero-padded lower, stop
        AB_MMS = [(1, 0, "pair"), (0, 4, "lower"), (2, 6, "upperz")]
        CD_MMS = [(1, 2, "pair"), (0, 5, "lower"), (2, 8, "upperz")]

        inv_n = 1.0 / (64.0 * 64.0)

        GC = GCHUNK  # chunks per repack group
        ngroups = NCHUNK // GC
        for p in range(2):  # batch pairs
            b0, b1 = 2 * p, 2 * p + 1
            acc = spool.tile([128, ngroups], F32, tag="acc")
            for g in range(ngroups):
                # qq[:, k, i, :] = [max(c00,c10) | max(c01,c11)], batch i
                qq = qpool.tile([128, GC, 2, 512], PPDT, tag="qq")
                for kk in range(GC):
                    m0 = 8 * (g * GC + kk)
                    for i, bbatch in enumerate((b0, b1)):
                        t = xt[bbatch]
                        psAB = pspool.tile([128, 512], F32, tag="ps")
                        psCD = pspool.tile([128, 512], F32, tag="ps")
                        for ps, mms, pp in ((psAB, AB_MMS, 0), (psCD, CD_MMS, 1)):
                            r0 = m0 + pp
                            for cw, s, mode in mms:
                                rhs = t[:, r0:r0 + 8, cw:cw + 64]
                                # partial-partition accumulation groups confuse
                                # the sim's group tracker; pending-zero still
                                # validates the data -> skip the group check.
                                if mode == "pair":
                                    nc.tensor.matmul(
                                        ps[:, :], wst2(s), rhs,
                                        start=True, stop=False,
                                        skip_group_check=True)
                                elif mode == "lower":
                                    nc.tensor.matmul(
                                        ps[0:64, :], wst1(s), rhs,
                                        start=False, stop=False,
                                        skip_group_check=True)
                                else:  # upperz
                                    nc.tensor.matmul(
                                        ps[:, :], wst2(s), rhs,
                                        start=False, stop=True,
                                        skip_group_check=True)
                        # DVE may read only one PSUM operand. Two schemes:
                        # MAXMODE psum: ScalarE copies CD; DVE maxes
                        #   PSUM(AB) x SBUF(cd) at 1x.
                        # MAXMODE sbuf: both banks copied to bf16 SBUF
                        #   (ScalarE mostly, DVE for every 4th) so the max
                        #   runs at bf16 2x on DVE.
                        cbidx = (g * GC + kk) * 2 + i
                        if PEONLY:
                            continue
                        if MAXMODE == "psum":
                            cd = cdpool.tile([128, 512], CDDT, tag="cd")
                            nc.scalar.copy(cd[:, :], psCD[:, :])
                            nc.vector.tensor_tensor(
                                qq[:, kk, i, :], psAB[:, :], cd[:, :], ALU.max)
                        else:
                            ca = cdpool.tile([128, 512], CDDT, tag="ca")
                            cd = cdpool.tile([128, 512], CDDT, tag="cd")
                            if cbidx % DVECOPY == DVECOPY - 1:
                                nc.vector.tensor_copy(ca[:, :], psAB[:, :])
                            else:
                                nc.scalar.copy(ca[:, :], psAB[:, :])
                            nc.scalar.copy(cd[:, :], psCD[:, :])
                            nc.vector.tensor_tensor(
                                qq[:, kk, i, :], ca[:, :], cd[:, :], ALU.max)
                if PEONLY:
                    continue
                # repack group to 128-lane batch-pair layout
                L = lrpool.tile([128, GC, 512], PPDT, tag="L")
                R = lrpool.tile([128, GC, 512], PPDT, tag="R")
                nc.sync.dma_start(L[0:64, :, :], qq[0:64, :, 0, :])
                nc.sync.dma_start(L[64:128, :, :], qq[0:64, :, 1, :])
                nc.sync.dma_start(R[0:64, :, :], qq[64:128, :, 0, :])
                nc.sync.dma_start(R[64:128, :, :], qq[64:128, :, 1, :])
                # clip R in place; then max(min(L,hi), R) in place over R;
                # accum_out = per-channel sum for this group
                te = getattr(nc, TAILENG)
                te.tensor_scalar(
                    R[:, :, :], R[:, :, :], hi, lo, ALU.min, ALU.max)
                te.scalar_tensor_tensor(
                    R[:, :, :], L[:, :, :], hi, R[:, :, :], ALU.min, ALU.max,
                    accum_out=acc[:, g:g + 1])
            S = spool.tile([128, 1], F32, tag="S")
            if PEONLY:
                nc.vector.tensor_copy(S[:, :], cs[:, 0:1])
            elif ngroups > 1:
                nc.vector.tensor_reduce(
                    S[:, :], acc[:, :], mybir.AxisListType.X, ALU.add)
            else:
                S = acc
            T = spool.tile([128, 1], F32, tag="T")
            nc.scalar.activation(
                T[:, :], S[:, :], mybir.ActivationFunctionType.Tanh,
                bias=bb, scale=inv_n)
            nc.sync.dma_start(out_d[2 * p:2 * p + 2, :], T[:, :])


def _round_tf32(a: np.ndarray) -> np.ndarray:
    """Round fp32 to tf32 (10-bit mantissa, round-to-nearest-even)."""
    u = np.ascontiguousarray(a, dtype=np.float32).view(np.uint32)
    bias = np.uint32(0xFFF) + ((u >> np.uint32(13)) & np.uint32(1))
    return ((u + bias) & np.uint32(0xFFFFE000)).view(np.float32)


def prep_core_inputs(x, w, b):
    """Host-side prep: pad/duplicate x, stationary-arrange w, fold b."""
    import ml_dtypes
    mmnp = ml_dtypes.bfloat16
    x = np.asarray(x, dtype=np.float32)
    w = np.asarray(w, dtype=np.float32)
    b = np.asarray(b, dtype=np.float32)

    ws = np.zeros((128, len(W_SLOTS), 64), np.float32)
    for s, j in enumerate(W_SLOTS):
        if j is None:
            continue
        kh, kw = j // 4, j % 4
        ws[0:64, s, :] = w[:, :, kh, kw]
        ws[64:128, s, :] = w[:, :, kh - 2, kw]
    ws = ws.astype(mmnp)

    cs = np.zeros((128, 3), np.float32)
    bd = np.concatenate([b, b])
    cs[:, 0] = 1.0 - bd
    cs[:, 1] = -1.0 - bd
    cs[:, 2] = bd

    in_maps = []
    for i in range(NCORES):
        xs = x[i * BPC:(i + 1) * BPC]
        xp = np.zeros((BPC, 128, PD, PD), np.float32)
        xp[:, 0:64, 1:65, 1:65] = xs    # dh=0 taps: P[r,s] = x[r-1,s-1]
        xp[:, 64:128, 0:64, 1:65] = xs  # dh=1 taps: shifted up one row
        in_maps.append({"xp": xp.astype(mmnp), "ws": ws, "cs": cs})
    return in_maps


class Runner:
    """Builds the 8-core shard_map'd executable once; callable many times
    (mirrors concourse.bass2jax.run_bass_via_pjrt)."""

    def __init__(self, nc=None):
        import jax
        from jax.sharding import Mesh, PartitionSpec, NamedSharding
        try:
            from jax.experimental.shard_map import shard_map
        except ImportError:
            from jax import shard_map
        from concourse.bass2jax import (
            _bass_exec_p, partition_id_tensor, install_neuronx_cc_hook)

        install_neuronx_cc_hook()
        self.nc = nc = nc if nc is not None else build_nc()
        pname = nc.partition_id_tensor.name if nc.partition_id_tensor else None
        in_names, out_names, out_avals, zero_outs = [], [], [], []
        for alloc in nc.m.functions[0].allocations:
            if not isinstance(alloc, mybir.MemoryLocationSet):
                continue
            name = alloc.memorylocations[0].name
            if alloc.kind == "ExternalInput":
                if name != pname:
                    in_names.append(name)
            elif alloc.kind == "ExternalOutput":
                out_names.append(name)
                shape = tuple(alloc.tensor_shape)
                dtype = mybir.dt.np(alloc.dtype)
                out_avals.append(jax.core.ShapedArray(shape, dtype))
                zero_outs.append(np.zeros(shape, dtype))
        self.in_names = list(in_names)
        self.out_names = out_names
        self.zero_outs = zero_outs
        n_params, n_outs = len(in_names), len(out_names)
        all_in = in_names + out_names + ([pname] if pname else [])

        def _body(*args):
            operands = list(args)
            if pname:
                operands.append(partition_id_tensor())
            return tuple(_bass_exec_p.bind(
                *operands,
                out_avals=tuple(out_avals),
                in_names=tuple(all_in),
                out_names=tuple(out_names),
                lowering_input_output_aliases=(),
                sim_require_finite=True,
                sim_require_nnan=True,
                nc=nc,
            ))

        devices = jax.devices()[:NCORES]
        self.mesh = Mesh(np.asarray(devices), ("core",))
        self.spec = PartitionSpec("core")
        self.sharding = NamedSharding(self.mesh, self.spec)
        in_specs = (self.spec,) * (n_params + n_outs)
        out_specs = (self.spec,) * n_outs
        self.fn = jax.jit(
            shard_map(_body, mesh=self.mesh, in_specs=in_specs,
                      out_specs=out_specs, check_rep=False),
            donate_argnums=tuple(range(n_params, n_params + n_outs)),
            keep_unused=True,
        )
        self._jax = jax

    def stage_inputs(self, in_maps):
        concat = [np.concatenate([np.asarray(m[n]) for m in in_maps], axis=0)
                  for n in self.in_names]
        return [self._jax.device_put(a, self.sharding) for a in concat]

    def __call__(self, staged):
        zeros = [np.zeros((NCORES * z.shape[0], *z.shape[1:]), z.dtype)
                 for z in self.zero_outs]
        return self.fn(*staged, *zeros)

    def run(self, in_maps):
        outs = self(self.stage_inputs(in_maps))
        return [
            {n: np.asarray(outs[i]).reshape(NCORES, *self.zero_outs[i].shape)[c]
             for i, n in enumerate(self.out_names)}
            for c in range(NCORES)
        ]


def kernel(x: np.ndarray, w: np.ndarray, b: np.ndarray) -> np.ndarray:
    in_maps = prep_core_inputs(x, w, b)
    try:
        # blessed entry point: handles both native (/dev/neuron*) and
        # axon-tunneled (PJRT) execution
        from concourse.bass_utils import run_bass_kernel_spmd
        nc = build_nc()
        res = run_bass_kernel_spmd(nc, in_maps, list(range(NCORES)))
        results = res.results
    except Exception:
        results = Runner().run(in_maps)
    out = np.concatenate([results[i]["out"] for i in range(NCORES)], axis=0)
    return out.reshape(B, C, 1, 1).astype(np.float32)


if __name__ == "__main__":
    rng = np.random.default_rng(0)
    x = rng.standard_normal((B, C, H, W), dtype=np.float32)
    w = rng.standard_normal((C, C, 4, 4), dtype=np.float32) * 0.05
    b = rng.standard_normal((C,), dtype=np.float32) * 0.05
    print(kernel(x, w, b).shape)
